# revision 17
# baseline (speedup 1.0000x reference)
"""Multi-head attention Trainium2 kernel (nn_MultiHeadAttention dense_transformer).

Reference computation (B=2, SQ=SK=2048, QDIM=KDIM=HID=1024, H=16, DH=64):
    qh = einsum('bsd,hde->bhse', q, Wq);  kh, vh likewise
    scores = qh @ kh^T / sqrt(DH);  scores[mask] = -inf
    att = softmax(scores) @ vh
    out = concat_heads(att) @ Wo

Sharding: 8 cores, core c owns batch b = c//4 and heads hs = (c%4)*4 .. hs+4.
Each core computes a partial output (its 4 heads' contribution to out[b]);
the host sums the 4 partials per batch.

Per-core device layout (transposed-scores / sT formulation):
  - host pre-transposes q/k/v/mask, so DMAs are contiguous
  - qhT/khT [64, SQ] computed pair-packed into [128, SQ] (head 2p in
    partitions 0:64, head 2p+1 in 64:128) via f32r matmuls
  - vh [SK, 64] per head, augmented with a ones column -> att matmul
    emits softmax row-sums for free in its 65th output row
  - scores^T [k, q] per 128x512 block -> exp (no max-subtraction: scores
    are O(5) by construction) -> multiply by not-mask -> att accumulation
  - normalization folded in before the output projection via a PE
    broadcast of the reciprocal row sums
"""

import sys

sys.path.insert(0, "/opt/trn_rl_repo")

import numpy as np
import ml_dtypes

BF16 = ml_dtypes.bfloat16

N_CORES = 8
B = 2
H = 16
DH = 64
NH = 4  # heads per core
NPAIR = 2  # head pairs per core

# Full-size problem dims (overridable for simulator-scale testing)
FULL = dict(SQ=2048, SK=2048, HID=1024)


def build_nc(SQ=2048, SK=2048, HID=1024, enable_asserts=False):
    """Build + compile the per-core Bass program (same program on all cores)."""
    import concourse.bacc as bacc
    import concourse.tile as tile
    from concourse import mybir

    f32 = mybir.dt.float32
    f32r = mybir.dt.float32r
    bf16 = mybir.dt.bfloat16

    HT = HID // 128  # hid k-tiles
    KB = SK // 128  # key blocks
    QC = SQ // 512  # query chunks (free dim of sT blocks)
    QB = SQ // 128  # query blocks (outproj M tiles)
    NCO = HID // 512  # outproj N chunks

    nc = bacc.Bacc(
        "TRN2", target_bir_lowering=False, debug=False, enable_asserts=enable_asserts
    )

    qT_d = nc.dram_tensor("qT", [HID, SQ], f32r, kind="ExternalInput")
    kT_d = nc.dram_tensor("kT", [HID, SK], f32r, kind="ExternalInput")
    vT_d = nc.dram_tensor("vT", [HID, SK], bf16, kind="ExternalInput")
    nmT_d = nc.dram_tensor("nmT", [SK, SQ], bf16, kind="ExternalInput")
    wq_d = nc.dram_tensor("wq", [HID, NH * DH], f32r, kind="ExternalInput")
    wk_d = nc.dram_tensor("wk", [HID, NH * DH], f32r, kind="ExternalInput")
    wv_d = nc.dram_tensor("wv", [HID, NH * DH], bf16, kind="ExternalInput")
    wo_d = nc.dram_tensor("wo", [NH * DH, HID], bf16, kind="ExternalInput")
    out_d = nc.dram_tensor("out", [SQ, HID], f32, kind="ExternalOutput")

    with tile.TileContext(nc) as tc:
        with (
            tc.tile_pool(name="consts", bufs=1) as consts,
            tc.tile_pool(name="xt", bufs=3) as xt_pool,
            tc.tile_pool(name="nm", bufs=3) as nm_pool,
            tc.tile_pool(name="attn", bufs=3) as attn_pool,
            tc.tile_pool(name="attu", bufs=2) as attu_pool,
            tc.tile_pool(name="small", bufs=1) as small_pool,
            tc.tile_pool(name="osb", bufs=3) as osb_pool,
        ):
            # ---- constant / persistent SBUF tensors ----
            wq_sb = consts.tile([128, HT, NH * DH], f32r)
            wk_sb = consts.tile([128, HT, NH * DH], f32r)
            wv_sb = consts.tile([128, HT, NH * DH], bf16)
            wo_sb = consts.tile([64, NH, HID], bf16)
            nc.sync.dma_start(
                wq_sb[:], wq_d[:, :].rearrange("(ht p) m -> p ht m", p=128)
            )
            nc.sync.dma_start(
                wk_sb[:], wk_d[:, :].rearrange("(ht p) m -> p ht m", p=128)
            )
            nc.sync.dma_start(
                wv_sb[:], wv_d[:, :].rearrange("(ht p) m -> p ht m", p=128)
            )
            nc.sync.dma_start(wo_sb[:], wo_d[:, :].rearrange("(h d) n -> d h n", d=64))

            ones_sb = consts.tile([65, 64], f32)
            nc.vector.memset(ones_sb[:], 1.0)
            ones_r = consts.tile([65, 64], f32r)
            nc.scalar.copy(ones_r[64:65, :], ones_sb[64:65, :])

            vh_sb = consts.tile([128, KB, NH, DH + 1], bf16)
            qhT_sb = consts.tile([128, NPAIR, SQ], f32r)
            khT_sb = consts.tile([128, NPAIR, SK], f32r)
            attT_sb = consts.tile([64, NH, SQ], bf16)

            # ---- phase 1: projections (PSUM pool: 8 x 1-bank slots) ----
            with tc.tile_pool(name="psP", bufs=8, space="PSUM") as psP:
                # v projection: vh[kb] [128k, NH*DH] += vT_tile^T @ wv.
                # Two half-passes of KB/2 key blocks so only 8 PSUM banks are
                # live; vT tiles are re-streamed per pass (extra 4MB DMA).
                KBH = KB // 2
                for half in range(2):
                    vh_ps_list = [
                        psP.tile([128, NH * DH], f32, tag="ps", name=f"vh_ps{half}_{i}")
                        for i in range(KBH)
                    ]
                    for ht in range(HT):
                        vt = xt_pool.tile(
                            [128, SK], bf16, tag="xt", name=f"vt{half}_{ht}"
                        )
                        nc.sync.dma_start(vt[:], vT_d[ht * 128 : (ht + 1) * 128, :])
                        for kbi in range(KBH):
                            kb = half * KBH + kbi
                            nc.tensor.matmul(
                                vh_ps_list[kbi][:],
                                vt[:, kb * 128 : (kb + 1) * 128],
                                wv_sb[:, ht, :],
                                start=(ht == 0),
                                stop=(ht == HT - 1),
                            )
                    for kbi in range(KBH):
                        kb = half * KBH + kbi
                        nc.vector.tensor_copy(
                            vh_sb[:, kb, :, 0:DH],
                            vh_ps_list[kbi][:].rearrange("p (h d) -> p h d", h=NH),
                        )
                        nc.vector.memset(vh_sb[:, kb, :, DH], 1.0)

                # q / k projections (f32r), pair-packed
                for which, x_d, w_sb, xh_sb in (
                    ("q", qT_d, wq_sb, qhT_sb),
                    ("k", kT_d, wk_sb, khT_sb),
                ):
                    S = SQ if which == "q" else SK
                    SC = S // 512
                    xh_ps = [
                        [
                            psP.tile(
                                [128, 512], f32, tag="ps", name=f"{which}h_ps{p}_{sc}"
                            )
                            for sc in range(SC)
                        ]
                        for p in range(NPAIR)
                    ]
                    for ht in range(HT):
                        xt = xt_pool.tile([128, S], f32r, tag="xt")
                        nc.sync.dma_start(xt[:], x_d[ht * 128 : (ht + 1) * 128, :])
                        for p in range(NPAIR):
                            for sc in range(SC):
                                nc.tensor.matmul(
                                    xh_ps[p][sc][:],
                                    w_sb[:, ht, p * 128 : (p + 1) * 128],
                                    xt[:, sc * 512 : (sc + 1) * 512],
                                    start=(ht == 0),
                                    stop=(ht == HT - 1),
                                )
                    for p in range(NPAIR):
                        for sc in range(SC):
                            nc.scalar.copy(
                                xh_sb[:, p, sc * 512 : (sc + 1) * 512], xh_ps[p][sc][:]
                            )

            # ---- phase 2: attention (PSUM: one 4-bank sT/bc slot + one
            #      4-bank att slot) ----
            with (
                tc.tile_pool(name="psA", bufs=1, space="PSUM") as psA,
                tc.tile_pool(name="psO", bufs=2, space="PSUM") as psO,
            ):
                for qc in range(QC):
                    qsl = slice(qc * 512, (qc + 1) * 512)
                    att_u = attu_pool.tile(
                        [65, NH, 512], f32, tag="attu", name=f"att_u{qc}"
                    )
                    # Pair-serial kb sweep: sT tiles are 2 banks with 3 slots,
                    # so the PE runs 2-3 iterations ahead of ScalarE's exp and
                    # never idles long enough for HAM to re-throttle. The mask
                    # is folded in on the PE: sT += ident^T @ (-30*maskT).
                    for p in range(NPAIR):
                        att_ps = psA.tile(
                            [65, 2, 512], f32, tag="att", name=f"att_ps{qc}_{p}"
                        )
                        for kb in range(KB):
                            nm_t = nm_pool.tile([128, 512], bf16, tag="nm")
                            nc.sync.dma_start(
                                nm_t[:], nmT_d[kb * 128 : (kb + 1) * 128, qsl]
                            )
                            sT_ps = psA.tile(
                                [128, 2, 512], f32, tag="st", bufs=2, name="sT"
                            )
                            for hh in range(2):
                                r = hh * 64
                                nc.tensor.matmul(
                                    sT_ps[:, hh, :],
                                    khT_sb[r : r + 64, p, kb * 128 : (kb + 1) * 128],
                                    qhT_sb[r : r + 64, p, qsl],
                                    start=True,
                                    stop=True,
                                )
                            attn_t = attn_pool.tile([128, 2, 512], bf16, tag="attn")
                            nc.scalar.activation(
                                attn_t[:], sT_ps[:], mybir.ActivationFunctionType.Exp
                            )
                            attn_m = attn_pool.tile([128, 2, 512], bf16, tag="attn")
                            nc.vector.tensor_mul(
                                attn_m[:],
                                attn_t[:],
                                nm_t[:].unsqueeze(1).broadcast_to((128, 2, 512)),
                            )
                            for hh in range(2):
                                h = 2 * p + hh
                                nc.tensor.matmul(
                                    att_ps[:, hh, :],
                                    vh_sb[:, kb, h, :],
                                    attn_m[:, hh, :],
                                    start=(kb == 0),
                                    stop=(kb == KB - 1),
                                )
                        nc.vector.tensor_copy(
                            att_u[:, 2 * p : 2 * p + 2, :], att_ps[:]
                        )
                    # normalize: attT[:, h, qc] = att * (1 / rowsum); rowsums
                    # sit in partition 64 (the vh ones-column output row).
                    # 1/sum = exp(-ln(sum)) on ScalarE; PE K=1 matmul
                    # broadcasts the reciprocals across partitions.
                    ln_t = small_pool.tile([65, NH, 512], f32, tag="ln")
                    nc.scalar.activation(
                        ln_t[64:65, :, :],
                        att_u[64:65, :, :],
                        mybir.ActivationFunctionType.Ln,
                    )
                    rc_t = small_pool.tile([65, NH, 512], f32r, tag="rc")
                    nc.scalar.activation(
                        rc_t[64:65, :, :],
                        ln_t[64:65, :, :],
                        mybir.ActivationFunctionType.Exp,
                        scale=-1.0,
                    )
                    for p in range(NPAIR):
                        bc_ps = psA.tile(
                            [64, 2, 512], f32, tag="st", bufs=2, name=f"bc{qc}_{p}"
                        )
                        for hh in range(2):
                            nc.tensor.matmul(
                                bc_ps[:, hh, :],
                                ones_r[64:65, :],
                                rc_t[64:65, 2 * p + hh, :],
                                start=True,
                                stop=True,
                            )
                        nc.vector.tensor_mul(
                            attT_sb[:, 2 * p : 2 * p + 2, qsl],
                            att_u[0:64, 2 * p : 2 * p + 2, :],
                            bc_ps[:],
                        )
                    # output projection for this qc's query blocks, overlapped
                    # with the next chunk's attention sweep
                    for qb in range(qc * (512 // 128), (qc + 1) * (512 // 128)):
                        for nco in range(NCO):
                            out_ps = psO.tile([128, 512], f32, tag="ps")
                            for h in range(NH):
                                nc.tensor.matmul(
                                    out_ps[:],
                                    attT_sb[:, h, qb * 128 : (qb + 1) * 128],
                                    wo_sb[:, h, nco * 512 : (nco + 1) * 512],
                                    start=(h == 0),
                                    stop=(h == NH - 1),
                                )
                            out_sb = osb_pool.tile([128, 512], f32, tag="osb")
                            nc.vector.tensor_copy(out_sb[:], out_ps[:])
                            nc.sync.dma_start(
                                out_d[
                                    qb * 128 : (qb + 1) * 128,
                                    nco * 512 : (nco + 1) * 512,
                                ],
                                out_sb[:],
                            )

    nc.compile()
    return nc


def make_in_maps(q, k, v, mask, Wq, Wk, Wv, Wo):
    """Host-side sharding: per-core input dict (batch b = c//4, heads (c%4)*4+...)."""
    # scores = (q@Wq) @ (k@Wk)^T / sqrt(DH): fold the 1/sqrt(DH) into Wq.
    dh = Wq.shape[-1]
    sc = 1.0 / np.sqrt(np.float32(dh))
    in_maps = []
    for c in range(N_CORES):
        b = c // (N_CORES // B)
        hs = (c % (N_CORES // B)) * NH
        qT = np.ascontiguousarray(q[b].T)
        kT = np.ascontiguousarray(k[b].T)
        vT = np.ascontiguousarray(v[b].T.astype(BF16))
        nmT = np.ascontiguousarray((~mask[b]).T).astype(BF16)
        wq = np.ascontiguousarray(
            (Wq[hs : hs + NH] * sc).transpose(1, 0, 2).reshape(Wq.shape[1], NH * dh)
        )
        wk = np.ascontiguousarray(
            Wk[hs : hs + NH].transpose(1, 0, 2).reshape(Wk.shape[1], NH * dh)
        )
        wv = np.ascontiguousarray(
            Wv[hs : hs + NH].transpose(1, 0, 2).reshape(Wv.shape[1], NH * dh)
        ).astype(BF16)
        wo = np.ascontiguousarray(Wo[hs * dh : (hs + NH) * dh, :]).astype(BF16)
        in_maps.append(
            {"qT": qT, "kT": kT, "vT": vT, "nmT": nmT,
             "wq": wq, "wk": wk, "wv": wv, "wo": wo}
        )
    return in_maps


_NC_CACHE = {}


def _get_nc():
    key = "full"
    if key not in _NC_CACHE:
        _NC_CACHE[key] = build_nc(**FULL)
    return _NC_CACHE[key]


def run_on_hw(nc, in_maps, **kwargs):
    from concourse.bass_utils import run_bass_kernel_spmd

    return run_bass_kernel_spmd(nc, in_maps, core_ids=list(range(N_CORES)), **kwargs)


def gather_output(results, q):
    B_, SQ_, QDIM_ = q.shape
    out = np.zeros((B_, SQ_, QDIM_), np.float32)
    for c in range(N_CORES):
        out[c // (N_CORES // B_)] += results[c]["out"]
    return out


def kernel(q, k, v, mask, Wq, Wk, Wv, Wo):
    nc = _get_nc()
    in_maps = make_in_maps(q, k, v, mask, Wq, Wk, Wv, Wo)
    res = run_on_hw(nc, in_maps)
    return gather_output(res.results, q)


# revision 18
# speedup vs baseline: 1.0145x; 1.0145x over previous
"""Multi-head attention Trainium2 kernel (nn_MultiHeadAttention dense_transformer).

Reference computation (B=2, SQ=SK=2048, QDIM=KDIM=HID=1024, H=16, DH=64):
    qh = einsum('bsd,hde->bhse', q, Wq);  kh, vh likewise
    scores = qh @ kh^T / sqrt(DH);  scores[mask] = -inf
    att = softmax(scores) @ vh
    out = concat_heads(att) @ Wo

Sharding: 8 cores, core c owns batch b = c//4 and heads hs = (c%4)*4 .. hs+4.
Each core computes a partial output (its 4 heads' contribution to out[b]);
the host sums the 4 partials per batch.

Per-core device layout (transposed-scores / sT formulation):
  - host pre-transposes q/k/v/mask, so DMAs are contiguous
  - qhT/khT [64, SQ] computed pair-packed into [128, SQ] (head 2p in
    partitions 0:64, head 2p+1 in 64:128) via f32r matmuls
  - vh [SK, 64] per head, augmented with a ones column -> att matmul
    emits softmax row-sums for free in its 65th output row
  - scores^T [k, q] per 128x512 block -> exp (no max-subtraction: scores
    are O(5) by construction) -> multiply by not-mask -> att accumulation
  - normalization folded in before the output projection via a PE
    broadcast of the reciprocal row sums
"""

import sys

sys.path.insert(0, "/opt/trn_rl_repo")

import numpy as np
import ml_dtypes

BF16 = ml_dtypes.bfloat16

N_CORES = 8
B = 2
H = 16
DH = 64
NH = 4  # heads per core
NPAIR = 2  # head pairs per core

# Full-size problem dims (overridable for simulator-scale testing)
FULL = dict(SQ=2048, SK=2048, HID=1024)


def build_nc(SQ=2048, SK=2048, HID=1024, enable_asserts=False):
    """Build + compile the per-core Bass program (same program on all cores)."""
    import concourse.bacc as bacc
    import concourse.tile as tile
    from concourse import mybir

    f32 = mybir.dt.float32
    f32r = mybir.dt.float32r
    bf16 = mybir.dt.bfloat16

    HT = HID // 128  # hid k-tiles
    KB = SK // 128  # key blocks
    QC = SQ // 512  # query chunks (free dim of sT blocks)
    QB = SQ // 128  # query blocks (outproj M tiles)
    NCO = HID // 512  # outproj N chunks

    nc = bacc.Bacc(
        "TRN2", target_bir_lowering=False, debug=False, enable_asserts=enable_asserts
    )

    qT_d = nc.dram_tensor("qT", [HID, SQ], f32r, kind="ExternalInput")
    kT_d = nc.dram_tensor("kT", [HID, SK], f32r, kind="ExternalInput")
    vT_d = nc.dram_tensor("vT", [HID, SK], bf16, kind="ExternalInput")
    nmT_d = nc.dram_tensor("nmT", [SK, SQ], bf16, kind="ExternalInput")
    wq_d = nc.dram_tensor("wq", [HID, NH * DH], f32r, kind="ExternalInput")
    wk_d = nc.dram_tensor("wk", [HID, NH * DH], f32r, kind="ExternalInput")
    wv_d = nc.dram_tensor("wv", [HID, NH * DH], bf16, kind="ExternalInput")
    wo_d = nc.dram_tensor("wo", [NH * DH, HID], bf16, kind="ExternalInput")
    out_d = nc.dram_tensor("out", [SQ, HID], f32, kind="ExternalOutput")

    with tile.TileContext(nc) as tc:
        with (
            tc.tile_pool(name="consts", bufs=1) as consts,
            tc.tile_pool(name="xt", bufs=3) as xt_pool,
            tc.tile_pool(name="nm", bufs=3) as nm_pool,
            tc.tile_pool(name="attn", bufs=3) as attn_pool,
            tc.tile_pool(name="attu", bufs=2) as attu_pool,
            tc.tile_pool(name="small", bufs=1) as small_pool,
            tc.tile_pool(name="osb", bufs=3) as osb_pool,
        ):
            # ---- constant / persistent SBUF tensors ----
            wq_sb = consts.tile([128, HT, NH * DH], f32r)
            wk_sb = consts.tile([128, HT, NH * DH], f32r)
            wv_sb = consts.tile([128, HT, NH * DH], bf16)
            wo_sb = consts.tile([128, NPAIR, HID], bf16)
            nc.sync.dma_start(
                wq_sb[:], wq_d[:, :].rearrange("(ht p) m -> p ht m", p=128)
            )
            nc.sync.dma_start(
                wk_sb[:], wk_d[:, :].rearrange("(ht p) m -> p ht m", p=128)
            )
            nc.sync.dma_start(
                wv_sb[:], wv_d[:, :].rearrange("(ht p) m -> p ht m", p=128)
            )
            nc.sync.dma_start(wo_sb[:], wo_d[:, :].rearrange("(p d) n -> d p n", d=128))

            ones_sb = consts.tile([65, 64], f32)
            nc.vector.memset(ones_sb[:], 1.0)
            ones_r = consts.tile([65, 64], f32r)
            nc.scalar.copy(ones_r[64:65, :], ones_sb[64:65, :])

            vh_sb = consts.tile([128, KB, NH, DH + 1], bf16)
            qhT_sb = consts.tile([128, NPAIR, SQ], f32r)
            khT_sb = consts.tile([128, NPAIR, SK], f32r)
            attT_sb = consts.tile([128, NPAIR, SQ], bf16)

            # ---- phase 1: projections (PSUM pool: 8 x 1-bank slots) ----
            with tc.tile_pool(name="psP", bufs=8, space="PSUM") as psP:
                # v projection: vh[kb] [128k, NH*DH] += vT_tile^T @ wv.
                # Two half-passes of KB/2 key blocks so only 8 PSUM banks are
                # live; vT tiles are re-streamed per pass (extra 4MB DMA).
                KBH = KB // 2
                for half in range(2):
                    vh_ps_list = [
                        psP.tile([128, NH * DH], f32, tag="ps", name=f"vh_ps{half}_{i}")
                        for i in range(KBH)
                    ]
                    for ht in range(HT):
                        vt = xt_pool.tile(
                            [128, SK], bf16, tag="xt", name=f"vt{half}_{ht}"
                        )
                        nc.sync.dma_start(vt[:], vT_d[ht * 128 : (ht + 1) * 128, :])
                        for kbi in range(KBH):
                            kb = half * KBH + kbi
                            nc.tensor.matmul(
                                vh_ps_list[kbi][:],
                                vt[:, kb * 128 : (kb + 1) * 128],
                                wv_sb[:, ht, :],
                                start=(ht == 0),
                                stop=(ht == HT - 1),
                            )
                    for kbi in range(KBH):
                        kb = half * KBH + kbi
                        nc.vector.tensor_copy(
                            vh_sb[:, kb, :, 0:DH],
                            vh_ps_list[kbi][:].rearrange("p (h d) -> p h d", h=NH),
                        )
                        nc.vector.memset(vh_sb[:, kb, :, DH], 1.0)

                # q / k projections (f32r), pair-packed
                for which, x_d, w_sb, xh_sb in (
                    ("q", qT_d, wq_sb, qhT_sb),
                    ("k", kT_d, wk_sb, khT_sb),
                ):
                    S = SQ if which == "q" else SK
                    SC = S // 512
                    xh_ps = [
                        [
                            psP.tile(
                                [128, 512], f32, tag="ps", name=f"{which}h_ps{p}_{sc}"
                            )
                            for sc in range(SC)
                        ]
                        for p in range(NPAIR)
                    ]
                    for ht in range(HT):
                        xt = xt_pool.tile([128, S], f32r, tag="xt")
                        nc.sync.dma_start(xt[:], x_d[ht * 128 : (ht + 1) * 128, :])
                        for p in range(NPAIR):
                            for sc in range(SC):
                                nc.tensor.matmul(
                                    xh_ps[p][sc][:],
                                    w_sb[:, ht, p * 128 : (p + 1) * 128],
                                    xt[:, sc * 512 : (sc + 1) * 512],
                                    start=(ht == 0),
                                    stop=(ht == HT - 1),
                                )
                    for p in range(NPAIR):
                        for sc in range(SC):
                            nc.scalar.copy(
                                xh_sb[:, p, sc * 512 : (sc + 1) * 512], xh_ps[p][sc][:]
                            )

            # ---- phase 2: attention (PSUM: one 4-bank sT/bc slot + one
            #      4-bank att slot) ----
            with tc.tile_pool(name="psA", bufs=1, space="PSUM") as psA:
                for qc in range(QC):
                    qsl = slice(qc * 512, (qc + 1) * 512)
                    att_u = attu_pool.tile(
                        [65, NH, 512], f32, tag="attu", name=f"att_u{qc}"
                    )
                    # Pair-serial kb sweep: sT tiles are 2 banks with 3 slots,
                    # so the PE runs 2-3 iterations ahead of ScalarE's exp and
                    # never idles long enough for HAM to re-throttle. The mask
                    # is folded in on the PE: sT += ident^T @ (-30*maskT).
                    for p in range(NPAIR):
                        att_ps = psA.tile(
                            [65, 2, 512], f32, tag="att", name=f"att_ps{qc}_{p}"
                        )
                        for kb in range(KB):
                            nm_t = nm_pool.tile([128, 512], bf16, tag="nm")
                            nc.sync.dma_start(
                                nm_t[:], nmT_d[kb * 128 : (kb + 1) * 128, qsl]
                            )
                            sT_ps = psA.tile(
                                [128, 2, 512], f32, tag="st", bufs=3, name="sT"
                            )
                            for hh in range(2):
                                r = hh * 64
                                nc.tensor.matmul(
                                    sT_ps[:, hh, :],
                                    khT_sb[r : r + 64, p, kb * 128 : (kb + 1) * 128],
                                    qhT_sb[r : r + 64, p, qsl],
                                    start=True,
                                    stop=True,
                                )
                            attn_t = attn_pool.tile([128, 2, 512], bf16, tag="attn")
                            nc.scalar.activation(
                                attn_t[:], sT_ps[:], mybir.ActivationFunctionType.Exp
                            )
                            attn_m = attn_pool.tile([128, 2, 512], bf16, tag="attn")
                            nc.vector.tensor_mul(
                                attn_m[:],
                                attn_t[:],
                                nm_t[:].unsqueeze(1).broadcast_to((128, 2, 512)),
                            )
                            for hh in range(2):
                                h = 2 * p + hh
                                nc.tensor.matmul(
                                    att_ps[:, hh, :],
                                    vh_sb[:, kb, h, :],
                                    attn_m[:, hh, :],
                                    start=(kb == 0),
                                    stop=(kb == KB - 1),
                                )
                        nc.vector.tensor_copy(
                            att_u[:, 2 * p : 2 * p + 2, :], att_ps[:]
                        )
                    # normalize: attT[:, h, qc] = att * (1 / rowsum); rowsums
                    # sit in partition 64 (the vh ones-column output row).
                    # 1/sum = exp(-ln(sum)) on ScalarE; PE K=1 matmul
                    # broadcasts the reciprocals across partitions.
                    ln_t = small_pool.tile([65, NH, 512], f32, tag="ln")
                    nc.scalar.activation(
                        ln_t[64:65, :, :],
                        att_u[64:65, :, :],
                        mybir.ActivationFunctionType.Ln,
                    )
                    rc_t = small_pool.tile([65, NH, 512], f32r, tag="rc")
                    nc.scalar.activation(
                        rc_t[64:65, :, :],
                        ln_t[64:65, :, :],
                        mybir.ActivationFunctionType.Exp,
                        scale=-1.0,
                    )
                    for p in range(NPAIR):
                        bc_ps = psA.tile(
                            [64, 2, 512], f32, tag="st", bufs=3, name=f"bc{qc}_{p}"
                        )
                        for hh in range(2):
                            nc.tensor.matmul(
                                bc_ps[:, hh, :],
                                ones_r[64:65, :],
                                rc_t[64:65, 2 * p + hh, :],
                                start=True,
                                stop=True,
                            )
                        for hh in range(2):
                            nc.vector.tensor_mul(
                                attT_sb[hh * 64 : hh * 64 + 64, p, qsl],
                                att_u[0:64, 2 * p + hh, :],
                                bc_ps[:, hh, :],
                            )

            # ---- phase 3: output projection, K=128 per head-pair ----
            with tc.tile_pool(name="psO", bufs=3, space="PSUM") as psO:
                for qb in range(QB):
                    for nco in range(NCO):
                        out_ps = psO.tile([128, 512], f32, tag="ps")
                        for p in range(NPAIR):
                            nc.tensor.matmul(
                                out_ps[:],
                                attT_sb[:, p, qb * 128 : (qb + 1) * 128],
                                wo_sb[:, p, nco * 512 : (nco + 1) * 512],
                                start=(p == 0),
                                stop=(p == NPAIR - 1),
                            )
                        out_sb = osb_pool.tile([128, 512], f32, tag="osb")
                        nc.vector.tensor_copy(out_sb[:], out_ps[:])
                        nc.sync.dma_start(
                            out_d[
                                qb * 128 : (qb + 1) * 128, nco * 512 : (nco + 1) * 512
                            ],
                            out_sb[:],
                        )


    nc.compile()
    return nc


def make_in_maps(q, k, v, mask, Wq, Wk, Wv, Wo):
    """Host-side sharding: per-core input dict (batch b = c//4, heads (c%4)*4+...)."""
    # scores = (q@Wq) @ (k@Wk)^T / sqrt(DH): fold the 1/sqrt(DH) into Wq.
    dh = Wq.shape[-1]
    sc = 1.0 / np.sqrt(np.float32(dh))
    in_maps = []
    for c in range(N_CORES):
        b = c // (N_CORES // B)
        hs = (c % (N_CORES // B)) * NH
        qT = np.ascontiguousarray(q[b].T)
        kT = np.ascontiguousarray(k[b].T)
        vT = np.ascontiguousarray(v[b].T.astype(BF16))
        nmT = np.ascontiguousarray((~mask[b]).T).astype(BF16)
        wq = np.ascontiguousarray(
            (Wq[hs : hs + NH] * sc).transpose(1, 0, 2).reshape(Wq.shape[1], NH * dh)
        )
        wk = np.ascontiguousarray(
            Wk[hs : hs + NH].transpose(1, 0, 2).reshape(Wk.shape[1], NH * dh)
        )
        wv = np.ascontiguousarray(
            Wv[hs : hs + NH].transpose(1, 0, 2).reshape(Wv.shape[1], NH * dh)
        ).astype(BF16)
        wo = np.ascontiguousarray(Wo[hs * dh : (hs + NH) * dh, :]).astype(BF16)
        in_maps.append(
            {"qT": qT, "kT": kT, "vT": vT, "nmT": nmT,
             "wq": wq, "wk": wk, "wv": wv, "wo": wo}
        )
    return in_maps


_NC_CACHE = {}


def _get_nc():
    key = "full"
    if key not in _NC_CACHE:
        _NC_CACHE[key] = build_nc(**FULL)
    return _NC_CACHE[key]


def run_on_hw(nc, in_maps, **kwargs):
    from concourse.bass_utils import run_bass_kernel_spmd

    return run_bass_kernel_spmd(nc, in_maps, core_ids=list(range(N_CORES)), **kwargs)


def gather_output(results, q):
    B_, SQ_, QDIM_ = q.shape
    out = np.zeros((B_, SQ_, QDIM_), np.float32)
    for c in range(N_CORES):
        out[c // (N_CORES // B_)] += results[c]["out"]
    return out


def kernel(q, k, v, mask, Wq, Wk, Wv, Wo):
    nc = _get_nc()
    in_maps = make_in_maps(q, k, v, mask, Wq, Wk, Wv, Wo)
    res = run_on_hw(nc, in_maps)
    return gather_output(res.results, q)


# revision 20
# speedup vs baseline: 1.1273x; 1.1113x over previous
"""Multi-head attention Trainium2 kernel (nn_MultiHeadAttention dense_transformer).

Reference computation (B=2, SQ=SK=2048, QDIM=KDIM=HID=1024, H=16, DH=64):
    qh = einsum('bsd,hde->bhse', q, Wq);  kh, vh likewise
    scores = qh @ kh^T / sqrt(DH);  scores[mask] = -inf
    att = softmax(scores) @ vh
    out = concat_heads(att) @ Wo

Sharding: 8 cores, core c owns batch b = c//4 and heads hs = (c%4)*4 .. hs+4.
Each core computes a partial output (its 4 heads' contribution to out[b]);
the host sums the 4 partials per batch.

Per-core device layout (transposed-scores / sT formulation):
  - host pre-transposes q/k/v/mask, so DMAs are contiguous
  - qhT/khT [64, SQ] computed pair-packed into [128, SQ] (head 2p in
    partitions 0:64, head 2p+1 in 64:128) via f32r matmuls
  - vh [SK, 64] per head, augmented with a ones column -> att matmul
    emits softmax row-sums for free in its 65th output row
  - scores^T [k, q] per 128x512 block -> exp (no max-subtraction: scores
    are O(5) by construction) -> multiply by not-mask -> att accumulation
  - normalization folded in before the output projection via a PE
    broadcast of the reciprocal row sums
"""

import sys

sys.path.insert(0, "/opt/trn_rl_repo")

import numpy as np
import ml_dtypes

BF16 = ml_dtypes.bfloat16

N_CORES = 8
B = 2
H = 16
DH = 64
NH = 4  # heads per core
NPAIR = 2  # head pairs per core

# Full-size problem dims (overridable for simulator-scale testing)
FULL = dict(SQ=2048, SK=2048, HID=1024)


def build_nc(SQ=2048, SK=2048, HID=1024, enable_asserts=False):
    """Build + compile the per-core Bass program (same program on all cores)."""
    import concourse.bacc as bacc
    import concourse.tile as tile
    from concourse import mybir

    f32 = mybir.dt.float32
    f32r = mybir.dt.float32r
    bf16 = mybir.dt.bfloat16

    HT = HID // 128  # hid k-tiles
    KB = SK // 128  # key blocks
    QC = SQ // 512  # query chunks (free dim of sT blocks)
    QB = SQ // 128  # query blocks (outproj M tiles)
    NCO = HID // 512  # outproj N chunks

    nc = bacc.Bacc(
        "TRN2", target_bir_lowering=False, debug=False, enable_asserts=enable_asserts
    )

    qT_d = nc.dram_tensor("qT", [HID, SQ], f32r, kind="ExternalInput")
    kT_d = nc.dram_tensor("kT", [HID, SK], f32r, kind="ExternalInput")
    vT_d = nc.dram_tensor("vT", [HID, SK], bf16, kind="ExternalInput")
    nmT_d = nc.dram_tensor("nmT", [SK, SQ], bf16, kind="ExternalInput")
    wq_d = nc.dram_tensor("wq", [HID, NH * DH], f32r, kind="ExternalInput")
    wk_d = nc.dram_tensor("wk", [HID, NH * DH], f32r, kind="ExternalInput")
    wv_d = nc.dram_tensor("wv", [HID, NH * DH], bf16, kind="ExternalInput")
    wo_d = nc.dram_tensor("wo", [NH * DH, HID], bf16, kind="ExternalInput")
    out_d = nc.dram_tensor("out", [SQ, HID], f32, kind="ExternalOutput")

    with tile.TileContext(nc) as tc:
        with (
            tc.tile_pool(name="consts", bufs=1) as consts,
            tc.tile_pool(name="xt", bufs=3) as xt_pool,
            tc.tile_pool(name="nm", bufs=4) as nm_pool,
            tc.tile_pool(name="attn", bufs=6) as attn_pool,
            tc.tile_pool(name="attu", bufs=2) as attu_pool,
            tc.tile_pool(name="small", bufs=1) as small_pool,
            tc.tile_pool(name="osb", bufs=3) as osb_pool,
        ):
            # ---- constant / persistent SBUF tensors ----
            wq_sb = consts.tile([128, HT, NH * DH], f32r)
            wk_sb = consts.tile([128, HT, NH * DH], f32r)
            wv_sb = consts.tile([128, HT, NH * DH], bf16)
            wo_sb = consts.tile([128, NPAIR, HID], bf16)
            nc.sync.dma_start(
                wq_sb[:], wq_d[:, :].rearrange("(ht p) m -> p ht m", p=128)
            )
            nc.sync.dma_start(
                wk_sb[:], wk_d[:, :].rearrange("(ht p) m -> p ht m", p=128)
            )
            nc.sync.dma_start(
                wv_sb[:], wv_d[:, :].rearrange("(ht p) m -> p ht m", p=128)
            )
            nc.sync.dma_start(wo_sb[:], wo_d[:, :].rearrange("(p d) n -> d p n", d=128))

            ones_sb = consts.tile([65, 64], f32)
            nc.vector.memset(ones_sb[:], 1.0)
            ones_r = consts.tile([65, 64], f32r)
            nc.scalar.copy(ones_r[64:65, :], ones_sb[64:65, :])

            vh_sb = consts.tile([128, KB, NH, DH + 1], bf16)
            qhT_sb = consts.tile([128, NPAIR, SQ], f32r)
            khT_sb = consts.tile([128, NPAIR, SK], f32r)
            attT_sb = consts.tile([128, NPAIR, SQ], bf16)

            # ---- phase 1: projections (PSUM pool: 8 x 1-bank slots) ----
            with tc.tile_pool(name="psP", bufs=8, space="PSUM") as psP:
                # v projection: vh[kb] [128k, NH*DH] += vT_tile^T @ wv.
                # Two half-passes of KB/2 key blocks so only 8 PSUM banks are
                # live; vT tiles are re-streamed per pass (extra 4MB DMA).
                KBH = KB // 2
                for half in range(2):
                    vh_ps_list = [
                        psP.tile([128, NH * DH], f32, tag="ps", name=f"vh_ps{half}_{i}")
                        for i in range(KBH)
                    ]
                    for ht in range(HT):
                        vt = xt_pool.tile(
                            [128, SK], bf16, tag="xt", name=f"vt{half}_{ht}"
                        )
                        nc.sync.dma_start(vt[:], vT_d[ht * 128 : (ht + 1) * 128, :])
                        for kbi in range(KBH):
                            kb = half * KBH + kbi
                            nc.tensor.matmul(
                                vh_ps_list[kbi][:],
                                vt[:, kb * 128 : (kb + 1) * 128],
                                wv_sb[:, ht, :],
                                start=(ht == 0),
                                stop=(ht == HT - 1),
                            )
                    for kbi in range(KBH):
                        kb = half * KBH + kbi
                        nc.vector.tensor_copy(
                            vh_sb[:, kb, :, 0:DH],
                            vh_ps_list[kbi][:].rearrange("p (h d) -> p h d", h=NH),
                        )
                        nc.vector.memset(vh_sb[:, kb, :, DH], 1.0)

                # q / k projections (f32r), pair-packed
                for which, x_d, w_sb, xh_sb in (
                    ("q", qT_d, wq_sb, qhT_sb),
                    ("k", kT_d, wk_sb, khT_sb),
                ):
                    S = SQ if which == "q" else SK
                    SC = S // 512
                    xh_ps = [
                        [
                            psP.tile(
                                [128, 512], f32, tag="ps", name=f"{which}h_ps{p}_{sc}"
                            )
                            for sc in range(SC)
                        ]
                        for p in range(NPAIR)
                    ]
                    for ht in range(HT):
                        xt = xt_pool.tile([128, S], f32r, tag="xt")
                        nc.sync.dma_start(xt[:], x_d[ht * 128 : (ht + 1) * 128, :])
                        for p in range(NPAIR):
                            for sc in range(SC):
                                nc.tensor.matmul(
                                    xh_ps[p][sc][:],
                                    w_sb[:, ht, p * 128 : (p + 1) * 128],
                                    xt[:, sc * 512 : (sc + 1) * 512],
                                    start=(ht == 0),
                                    stop=(ht == HT - 1),
                                )
                    for p in range(NPAIR):
                        for sc in range(SC):
                            nc.scalar.copy(
                                xh_sb[:, p, sc * 512 : (sc + 1) * 512], xh_ps[p][sc][:]
                            )

            # ---- phase 2: attention (PSUM: one 4-bank sT/bc slot + one
            #      4-bank att slot) ----
            with tc.tile_pool(name="psA", bufs=1, space="PSUM") as psA:
                for qc in range(QC):
                    qsl = slice(qc * 512, (qc + 1) * 512)
                    att_u = attu_pool.tile(
                        [65, NH, 512], f32, tag="attu", name=f"att_u{qc}"
                    )
                    # Pair-serial kb sweep: sT tiles are 2 banks with 3 slots,
                    # so the PE runs 2-3 iterations ahead of ScalarE's exp and
                    # never idles long enough for HAM to re-throttle. The mask
                    # is folded in on the PE: sT += ident^T @ (-30*maskT).
                    for p in range(NPAIR):
                        att_ps = psA.tile(
                            [65, 2, 512], f32, tag="att", name=f"att_ps{qc}_{p}"
                        )
                        for kb in range(KB):
                            nm_t = nm_pool.tile([128, 512], bf16, tag="nm")
                            nc.sync.dma_start(
                                nm_t[:], nmT_d[kb * 128 : (kb + 1) * 128, qsl]
                            )
                            sT_ps = psA.tile(
                                [128, 2, 512], f32, tag="st", bufs=3, name="sT"
                            )
                            for hh in range(2):
                                r = hh * 64
                                nc.tensor.matmul(
                                    sT_ps[:, hh, :],
                                    khT_sb[r : r + 64, p, kb * 128 : (kb + 1) * 128],
                                    qhT_sb[r : r + 64, p, qsl],
                                    start=True,
                                    stop=True,
                                )
                            attn_t = attn_pool.tile([128, 2, 512], bf16, tag="attn")
                            nc.scalar.activation(
                                attn_t[:], sT_ps[:], mybir.ActivationFunctionType.Exp
                            )
                            attn_m = attn_pool.tile([128, 2, 512], bf16, tag="attn")
                            nc.vector.tensor_mul(
                                attn_m[:],
                                attn_t[:],
                                nm_t[:].unsqueeze(1).broadcast_to((128, 2, 512)),
                            )
                            for hh in range(2):
                                h = 2 * p + hh
                                nc.tensor.matmul(
                                    att_ps[:, hh, :],
                                    vh_sb[:, kb, h, :],
                                    attn_m[:, hh, :],
                                    start=(kb == 0),
                                    stop=(kb == KB - 1),
                                )
                        nc.vector.tensor_copy(
                            att_u[:, 2 * p : 2 * p + 2, :], att_ps[:]
                        )
                    # normalize: attT[:, h, qc] = att * (1 / rowsum); rowsums
                    # sit in partition 64 (the vh ones-column output row).
                    # 1/sum = exp(-ln(sum)) on ScalarE; PE K=1 matmul
                    # broadcasts the reciprocals across partitions.
                    ln_t = small_pool.tile([65, NH, 512], f32, tag="ln")
                    nc.scalar.activation(
                        ln_t[64:65, :, :],
                        att_u[64:65, :, :],
                        mybir.ActivationFunctionType.Ln,
                    )
                    rc_t = small_pool.tile([65, NH, 512], f32r, tag="rc")
                    nc.scalar.activation(
                        rc_t[64:65, :, :],
                        ln_t[64:65, :, :],
                        mybir.ActivationFunctionType.Exp,
                        scale=-1.0,
                    )
                    for p in range(NPAIR):
                        bc_ps = psA.tile(
                            [64, 2, 512], f32, tag="st", bufs=3, name=f"bc{qc}_{p}"
                        )
                        for hh in range(2):
                            nc.tensor.matmul(
                                bc_ps[:, hh, :],
                                ones_r[64:65, :],
                                rc_t[64:65, 2 * p + hh, :],
                                start=True,
                                stop=True,
                            )
                        for hh in range(2):
                            nc.vector.tensor_mul(
                                attT_sb[hh * 64 : hh * 64 + 64, p, qsl],
                                att_u[0:64, 2 * p + hh, :],
                                bc_ps[:, hh, :],
                            )

            # ---- phase 3: output projection, K=128 per head-pair ----
            with tc.tile_pool(name="psO", bufs=3, space="PSUM") as psO:
                for qb in range(QB):
                    out_ps = psO.tile([128, NCO, 512], f32, tag="ps")
                    for nco in range(NCO):
                        for p in range(NPAIR):
                            nc.tensor.matmul(
                                out_ps[:, nco, :],
                                attT_sb[:, p, qb * 128 : (qb + 1) * 128],
                                wo_sb[:, p, nco * 512 : (nco + 1) * 512],
                                start=(p == 0),
                                stop=(p == NPAIR - 1),
                            )
                    out_sb = osb_pool.tile([128, NCO, 512], f32, tag="osb")
                    if qb % 2 == 0:
                        nc.vector.tensor_copy(out_sb[:], out_ps[:])
                    else:
                        nc.scalar.copy(out_sb[:], out_ps[:])
                    nc.sync.dma_start(
                        out_d[qb * 128 : (qb + 1) * 128, :],
                        out_sb[:].rearrange("p nco n -> p (nco n)"),
                    )


    nc.compile()
    return nc


def make_in_maps(q, k, v, mask, Wq, Wk, Wv, Wo):
    """Host-side sharding: per-core input dict (batch b = c//4, heads (c%4)*4+...)."""
    # scores = (q@Wq) @ (k@Wk)^T / sqrt(DH): fold the 1/sqrt(DH) into Wq.
    dh = Wq.shape[-1]
    sc = 1.0 / np.sqrt(np.float32(dh))
    in_maps = []
    for c in range(N_CORES):
        b = c // (N_CORES // B)
        hs = (c % (N_CORES // B)) * NH
        qT = np.ascontiguousarray(q[b].T)
        kT = np.ascontiguousarray(k[b].T)
        vT = np.ascontiguousarray(v[b].T.astype(BF16))
        nmT = np.ascontiguousarray((~mask[b]).T).astype(BF16)
        wq = np.ascontiguousarray(
            (Wq[hs : hs + NH] * sc).transpose(1, 0, 2).reshape(Wq.shape[1], NH * dh)
        )
        wk = np.ascontiguousarray(
            Wk[hs : hs + NH].transpose(1, 0, 2).reshape(Wk.shape[1], NH * dh)
        )
        wv = np.ascontiguousarray(
            Wv[hs : hs + NH].transpose(1, 0, 2).reshape(Wv.shape[1], NH * dh)
        ).astype(BF16)
        wo = np.ascontiguousarray(Wo[hs * dh : (hs + NH) * dh, :]).astype(BF16)
        in_maps.append(
            {"qT": qT, "kT": kT, "vT": vT, "nmT": nmT,
             "wq": wq, "wk": wk, "wv": wv, "wo": wo}
        )
    return in_maps


_NC_CACHE = {}


def _get_nc():
    key = "full"
    if key not in _NC_CACHE:
        _NC_CACHE[key] = build_nc(**FULL)
    return _NC_CACHE[key]


def run_on_hw(nc, in_maps, **kwargs):
    from concourse.bass_utils import run_bass_kernel_spmd

    return run_bass_kernel_spmd(nc, in_maps, core_ids=list(range(N_CORES)), **kwargs)


def gather_output(results, q):
    B_, SQ_, QDIM_ = q.shape
    out = np.zeros((B_, SQ_, QDIM_), np.float32)
    for c in range(N_CORES):
        out[c // (N_CORES // B_)] += results[c]["out"]
    return out


def kernel(q, k, v, mask, Wq, Wk, Wv, Wo):
    nc = _get_nc()
    in_maps = make_in_maps(q, k, v, mask, Wq, Wk, Wv, Wo)
    res = run_on_hw(nc, in_maps)
    return gather_output(res.results, q)
